# revision 30
# baseline (speedup 1.0000x reference)
"""Distributed Trainium2 kernel for a dense transformer block.

Reference computation (per batch):
  x = x + o_proj(attn(rope(qkv(rmsnorm(x))), causal))
  x = x + w2(silu(wg(rmsnorm(x))) * w1(rmsnorm(x)))

Sharding: DP=2 on batch x TP=4 on heads for attention.  After attention each
rank computes the PARTIAL o-proj (its own 256 head-features) for all tokens
of the chunk plus x/4, and a per-chunk ReduceScatter (two 256-token halves)
both completes the sum and hands every rank ownership of 128 tokens per
chunk.  Each rank then runs norm2 + the FULL-hidden MLP on its own 512
tokens and writes the final output shard directly -- only one collective
phase in the whole kernel.

Schedule (v3): fully software-pipelined.
  - rmsnorm statistics for all 16 token tiles are hoisted up front so the ACT
    engine runs only Exp during attention (activation-table switches cost
    1.3us each); the per-chunk normalize+transpose work is interleaved.
  - qkv(qt+1) and o-proj(qt-1) streams are interleaved into the attention(qt)
    score/AV stream so the PE never idles (HAM stays warm).
  - score/exp/AV streams are trimmed to the causal triangle at 128-column
    granularity.
"""

import sys

sys.path.insert(0, "/opt/trn_rl_repo")

import numpy as np
import ml_dtypes

import concourse.bass as bass
import concourse.bacc as bacc
import concourse.mybir as mybir
import concourse.tile as tile
from concourse.bass_utils import run_bass_kernel_spmd

BF = ml_dtypes.bfloat16
F32 = mybir.dt.float32
BF16 = mybir.dt.bfloat16

D = 1024
NH = 16
DH = 64
MULT = 4
EPS = 1e-5
ROPE_BASE = 10000.0
B = 2
TP = 4  # tensor-parallel ranks per group
HPC = NH // TP  # heads per core = 4
QKF = 2 * HPC * DH  # q+k shard features = 512
VF = HPC * DH  # v shard features = 256
MIDF = MULT * D  # full mlp hidden = 4096
HCN = MIDF // 128  # mlp hidden chunks = 32
AF = mybir.ActivationFunctionType
ALU = mybir.AluOpType


def interleave(*gens):
    """Round-robin a set of emission generators until all are exhausted."""
    active = [g for g in gens if g is not None]
    while active:
        nxt = []
        for g in active:
            try:
                next(g)
                nxt.append(g)
            except StopIteration:
                pass
        active = nxt


def delayed(gen, n):
    """Generator wrapper: yield n empty steps before starting gen."""
    for _ in range(n):
        yield
    yield from gen


def paced(gen, p):
    """Spread gen's steps over p rounds each, so a short filler generator
    covers a long main generator instead of exhausting early."""
    while True:
        try:
            next(gen)
        except StopIteration:
            return
        yield
        for _ in range(p - 1):
            yield


def build_nc(T):
    """Build the SPMD graph for one core (token count T per batch)."""
    DC = D // 128  # d chunks = 8
    TT = T // 128  # token tiles = 16
    QT = 512  # q-tile width == collective chunk width
    NQ = T // QT  # 4
    CPQ = QT // 128  # 4
    OWN = NQ * 128  # own tokens per core = 512

    nc = bacc.Bacc("TRN2", target_bir_lowering=False, debug=False, num_devices=8)

    x_e = nc.dram_tensor("x", [T, D], BF16, kind="ExternalInput")
    xf_e = nc.dram_tensor("x_fm", [D, T], BF16, kind="ExternalInput")
    qkw_e = nc.dram_tensor("qkw_t", [D, QKF], BF16, kind="ExternalInput")
    vw_e = nc.dram_tensor("vw_m", [D, VF], BF16, kind="ExternalInput")
    ow_e = nc.dram_tensor("ow_m", [VF, D], BF16, kind="ExternalInput")
    w1w_e = nc.dram_tensor("w1w_t", [D, MIDF], BF16, kind="ExternalInput")
    wgw_e = nc.dram_tensor("wgw_t", [D, MIDF], BF16, kind="ExternalInput")
    w2w_e = nc.dram_tensor("w2w_m", [MIDF, D], BF16, kind="ExternalInput")
    cos_e = nc.dram_tensor("cosr", [128, T], BF16, kind="ExternalInput")
    sin_e = nc.dram_tensor("sinr", [128, T], BF16, kind="ExternalInput")
    cm_e = nc.dram_tensor("cmask", [128, 128], BF16, kind="ExternalInput")
    id_e = nc.dram_tensor("ident", [128, 128], BF16, kind="ExternalInput")
    out_e = nc.dram_tensor("out", [OWN, D], F32, kind="ExternalOutput")

    groups = [[0, 1, 2, 3], [4, 5, 6, 7]]

    with tile.TileContext(nc) as tc:
        with (
            tc.tile_pool(name="const", bufs=1) as cpool,
            tc.tile_pool(name="actfm", bufs=1) as fmpool,
            tc.tile_pool(name="qko", bufs=1) as qkpool,
            tc.tile_pool(name="vaug", bufs=1) as vpool,
            tc.tile_pool(name="xnb", bufs=2) as xnpool,
            tc.tile_pool(name="work", bufs=4) as wpool,
            tc.tile_pool(name="rope", bufs=2) as rpool,
            tc.tile_pool(name="stats", bufs=8) as spool,
            tc.tile_pool(name="hres", bufs=1) as hpool,
            tc.tile_pool(name="dram", bufs=1, space="DRAM") as dpool,
        ):
            # ---- resident weights / tables ----
            def load_tiles(src, width, n, dt=BF16):
                ts = []
                for i in range(n):
                    t = cpool.tile(
                        [128, width], dt, tag=f"{src.name}_{i}", name=f"{src.name}_{i}"
                    )
                    nc.sync.dma_start(t[:], src[i * 128 : (i + 1) * 128, :])
                    ts.append(t)
                return ts

            ident = load_tiles(id_e, 128, 1)[0]
            epsc = cpool.tile([128, 1], F32, tag="epsc", name="epsc")
            nc.vector.memset(epsc[:], EPS)
            onesD = cpool.tile([128, 1], BF16, tag="onesD", name="onesD")
            nc.vector.memset(onesD[:], 1.0)
            ones128r = cpool.tile([1, 128], BF16, tag="ones128r", name="ones128r")
            nc.vector.memset(ones128r[:], 1.0)

            # feature-major x chunks (for norm1); 3 bufs so chunk t4+2's DMA
            # only WAR-waits on chunk t4-1's (already emitted) consumers
            xfr = xf_e.rearrange("(c p) t -> p c t", p=128)
            xf_c = [
                fmpool.tile(
                    [128, DC, QT], BF16, tag="xfm", name=f"xf{t4}", bufs=3
                )
                for t4 in range(NQ)
            ]
            nc.sync.dma_start(xf_c[0][:], xfr[:, :, 0:QT])

            qkw = load_tiles(qkw_e, QKF, DC)
            vw = load_tiles(vw_e, VF, DC)
            ow = load_tiles(ow_e, D, 2)  # local-feature o-proj rows
            w1r = w1w_e.rearrange("(c p) m -> p c m", p=128)
            wgr = wgw_e.rearrange("(c p) m -> p c m", p=128)
            cos_t, sin_t = [], []
            for t4 in range(NQ):
                for src, dst in ((cos_e, cos_t), (sin_e, sin_t)):
                    t = cpool.tile(
                        [128, QT], BF16, tag=f"{src.name}_{t4}",
                        name=f"{src.name}c{t4}",
                    )
                    nc.sync.dma_start(t[:], src[:, t4 * QT : (t4 + 1) * QT])
                    dst.append(t)
            tri = load_tiles(cm_e, 128, 1)[0]
            ones64 = cpool.tile([1, 64], BF16, tag="ones64", name="ones64")
            nc.vector.memset(ones64[:], 1.0)

            # remaining feature-major x chunks
            for t4 in range(1, NQ):
                nc.sync.dma_start(
                    xf_c[t4][:], xfr[:, :, t4 * QT : (t4 + 1) * QT]
                )
            # token-major x (for the o-proj residual): 8-tile sliding
            # window; chunks 2-3 are prefetched from inside oproj_gen
            x_t = [None] * TT

            def load_x(ti):
                t = cpool.tile(
                    [128, D], BF16, tag=f"x{ti % 8}", name=f"x{ti}"
                )
                nc.sync.dma_start(t[:], x_e[ti * 128 : (ti + 1) * 128, :])
                x_t[ti] = t

            for ti in range(8):
                load_x(ti)

            rs_in = [
                dpool.tile([QT, D], BF16, name=f"rs_in{k}") for k in range(NQ)
            ]
            rs_out = [
                dpool.tile([QT // TP, D], BF16, name=f"rs_out{k}")
                for k in range(NQ)
            ]

            # ---- persistent activation tiles ----
            # normalized x, feature-major: xnf[t4][:, dc, tau*128:...] holds
            # (x-hat chunk)^T for d-block dc
            xnf_c = [
                fmpool.tile(
                    [128, DC, QT], BF16, tag="fm", name=f"xnf{t4}", bufs=2
                )
                for t4 in range(NQ)
            ]
            q_sb = [
                qkpool.tile([128, T], BF16, tag=f"qk{i}", name=f"q{i}")
                for i in range(2)
            ]
            k_sb = [
                qkpool.tile([128, T], BF16, tag=f"qk{i + 2}", name=f"k{i}")
                for i in range(2)
            ]
            v_aug = [
                vpool.tile([128, HPC, DH + 1], BF16, tag=f"va{ti}", name=f"va{ti}")
                for ti in range(TT)
            ]
            hres = [
                hpool.tile([128, D], BF16, tag=f"hr{k}", name=f"hr{k}")
                for k in range(NQ)
            ]
            # normalized attention outputs per chunk+headpair (o-proj lhsT)
            On_store = [[None, None] for _ in range(NQ)]
            # norm1 1/rms rows (per chunk) + norm2 scalars (per chunk)
            s1r_t = [
                cpool.tile([1, QT], BF16, tag=f"s1r{t4}", name=f"s1r{t4}")
                for t4 in range(NQ)
            ]
            s2_t = [
                cpool.tile([128, 1], F32, tag=f"s2_{k}", name=f"s2_{k}")
                for k in range(NQ)
            ]

            def stats_tile(xt, s1):
                """rms stats for one token tile -> s1 = 1/sqrt(mean sq+eps)."""
                ss = spool.tile([128, 1], F32, tag="ss", name="ss")
                sq = xnpool.tile([128, D], BF16, tag="sq", name="sq", bufs=1)
                nc.vector.scalar_tensor_tensor(
                    sq[:], xt[:], 1.0, xt[:], ALU.mult, ALU.mult, accum_out=ss[:]
                )
                sr = spool.tile([128, 1], F32, tag="sr", name="sr")
                nc.scalar.activation(
                    out=sr[:], in_=ss[:], func=AF.Sqrt, bias=epsc[:], scale=1.0 / D
                )
                nc.vector.reciprocal(s1[:], sr[:])

            eps1 = spool.tile([1, 1], F32, tag="eps1", name="eps1")
            nc.vector.memset(eps1[:], EPS)

            with (
                tc.tile_pool(name="psSC", bufs=2, space="PSUM") as psSC,
                tc.tile_pool(name="psAV", bufs=2, space="PSUM") as psAV,
                tc.tile_pool(name="psQ", bufs=2, space="PSUM") as psQ,
            ):

                def apply_norm_gen(xt, s1, fm_c, tau, on_act=False):
                    """normalize token tile and write feature-major block."""
                    xn = xnpool.tile([128, D], BF16, tag="xn", name="xn", bufs=1)
                    nc.vector.tensor_scalar_mul(xn[:], xt[:], s1[:])
                    for di in range(DC):
                        tp = psQ.tile([128, 128], BF16, tag="q", name="tp")
                        nc.tensor.transpose(
                            tp[:], xn[:, di * 128 : (di + 1) * 128], ident[:]
                        )
                        dst = fm_c[:, di, tau * 128 : (tau + 1) * 128]
                        if on_act:
                            nc.scalar.copy(dst, tp[:])
                        else:
                            nc.vector.tensor_copy(dst, tp[:])
                        if di % 4 == 3:
                            yield

                # ---- qkv + rope + v for one chunk (generator) ----
                def qkv_gen(t4):
                    tsl = slice(t4 * QT, (t4 + 1) * QT)
                    for m in range(4):  # q01 q23 k01 k23
                        dst = q_sb[m] if m < 2 else k_sb[m - 2]
                        ps = psQ.tile([128, QT], F32, tag="q", name="ps")
                        for dc in range(DC):
                            nc.tensor.matmul(
                                ps[:],
                                qkw[dc][:, m * 128 : (m + 1) * 128],
                                xnf_c[t4][:, dc, :],
                                start=(dc == 0),
                                stop=(dc == DC - 1),
                            )
                            if dc == 3:
                                yield
                        qb = rpool.tile([128, QT], BF16, tag="qb", name="qb")
                        nc.vector.tensor_copy(qb[:], ps[:])
                        rot = rpool.tile([128, QT], BF16, tag="rot", name="rot")
                        for hb in (0, 64):
                            nc.vector.tensor_scalar_mul(
                                rot[hb : hb + 32, :], qb[hb + 32 : hb + 64, :], -1.0
                            )
                            nc.vector.tensor_copy(
                                rot[hb + 32 : hb + 64, :], qb[hb : hb + 32, :]
                            )
                        t1 = rpool.tile([128, QT], BF16, tag="t1", name="t1")
                        nc.vector.tensor_mul(t1[:], qb[:], cos_t[t4][:])
                        t2 = rpool.tile([128, QT], BF16, tag="t2", name="t2")
                        nc.vector.tensor_mul(t2[:], rot[:], sin_t[t4][:])
                        nc.vector.tensor_add(dst[:, tsl], t1[:], t2[:])
                        yield
                    for tau in range(CPQ):
                        ti = t4 * CPQ + tau
                        ps = psQ.tile([128, VF], F32, tag="q", name="psv")
                        for dc in range(DC):
                            nc.tensor.matmul(
                                ps[:],
                                xnf_c[t4][:, dc, tau * 128 : (tau + 1) * 128],
                                vw[dc][:],
                                start=(dc == 0),
                                stop=(dc == DC - 1),
                            )
                        va = v_aug[ti]
                        nc.vector.tensor_copy(
                            va[:, :, 0:DH], ps.rearrange("p (h d) -> p h d", h=HPC)
                        )
                        nc.vector.memset(va[:, :, DH : DH + 1], 1.0)
                        yield

                def stats_fm(t4):
                    """norm1 stats for one chunk from feature-major x:
                    s1r[t4][0, t] = 1/sqrt(mean_d x[t,d]^2 + eps)."""
                    xsq = xnpool.tile(
                        [128, DC, QT], BF16, tag="xsq", name="xsq", bufs=1
                    )
                    nc.vector.tensor_mul(xsq[:], xf_c[t4][:], xf_c[t4][:])
                    ssr = psQ.tile([1, QT], F32, tag="q", name="ssr")
                    for dc in range(DC):
                        nc.tensor.matmul(
                            ssr[:],
                            onesD[:],
                            xsq[:, dc, :],
                            start=(dc == 0),
                            stop=(dc == DC - 1),
                        )
                    srr = spool.tile([1, QT], F32, tag="srr", name="srr", bufs=2)
                    nc.scalar.activation(
                        out=srr[:], in_=ssr[:], func=AF.Sqrt, bias=eps1[:],
                        scale=1.0 / D,
                    )
                    with nc.allow_low_precision(reason="1/rms in bf16"):
                        nc.vector.reciprocal(s1r_t[t4][:], srr[:])

                def chunk_gen(t4):
                    """norm1-apply (via row broadcast) then qkv for chunk
                    t4 (chained)."""
                    bc = psQ.tile([128, QT], F32, tag="q", name="bc")
                    nc.tensor.matmul(
                        bc[:], ones128r[:], s1r_t[t4][:], start=True, stop=True
                    )
                    bcs = xnpool.tile([128, QT], BF16, tag="bcs", name="bcs", bufs=2)
                    nc.vector.tensor_copy(bcs[:], bc[:])
                    yield
                    for dc in range(DC):
                        nc.vector.tensor_mul(
                            xnf_c[t4][:, dc, :], xf_c[t4][:, dc, :], bcs[:]
                        )
                        if dc % 4 == 3:
                            yield
                    yield from qkv_gen(t4)

                # ---- attention for one q-chunk (generator) ----
                def attn_gen(qt):
                    ncks = CPQ * (qt + 1)
                    for hp in range(2):
                        opsP = [
                            psAV.tile([DH + 1, QT], F32, tag="av", name=f"ops{i}")
                            for i in range(2)
                        ]

                        def emit_scores(ck):
                            j = ck - CPQ * qt  # >=0 inside the diagonal block
                            lo = max(j, 0) * 128
                            sp = psSC.tile([128, 2, QT], F32, tag="sc", name="sp")
                            for i in range(2):
                                hb = i * 64
                                nc.tensor.matmul(
                                    sp[:, i, lo:QT],
                                    k_sb[hp][
                                        hb : hb + DH, ck * 128 : (ck + 1) * 128
                                    ],
                                    q_sb[hp][
                                        hb : hb + DH,
                                        qt * QT + lo : (qt + 1) * QT,
                                    ],
                                    start=True,
                                    stop=True,
                                )
                            pt = wpool.tile(
                                [128, 2, QT], BF16, tag="pt", name="pt", bufs=5
                            )
                            nc.scalar.activation(
                                out=pt[:, :, lo:],
                                in_=sp[:, :, lo:],
                                func=AF.Exp,
                                scale=0.125,
                            )
                            if j >= 0:
                                for i in range(2):
                                    nc.vector.tensor_mul(
                                        pt[:, i, lo : lo + 128],
                                        pt[:, i, lo : lo + 128],
                                        tri[:],
                                    )
                            return pt, lo

                        def emit_av(ck, pt_lo):
                            pt, lo = pt_lo
                            for i in range(2):
                                nc.tensor.matmul(
                                    opsP[i][:, lo:QT],
                                    v_aug[ck][:, 2 * hp + i, :],
                                    pt[:, i, lo:],
                                    start=(ck == 0),
                                    stop=(ck == ncks - 1),
                                )

                        # scores emitted one chunk ahead of AV so the PE
                        # never waits on the Exp chain
                        prev = emit_scores(0)
                        yield
                        for ck in range(1, ncks):
                            cur = emit_scores(ck)
                            emit_av(ck - 1, prev)
                            prev = cur
                            yield
                        emit_av(ncks - 1, prev)
                        yield
                        # copy raw head outputs + denominator rows out of
                        # PSUM fast, freeing the AV accumulators; both heads
                        # stack into one 128-partition tile so the later
                        # SBUF-only ops have uniform start partitions
                        Or = wpool.tile(
                            [128, QT], BF16, tag="Or", name="Or", bufs=4
                        )
                        dn = spool.tile(
                            [1, 2, QT], BF16, tag="dn", name="dn", bufs=2
                        )
                        for i in range(2):
                            nc.vector.tensor_copy(
                                Or[i * 64 : (i + 1) * 64, :], opsP[i][0:DH, :]
                            )
                            nc.vector.tensor_copy(
                                dn[:, i, :], opsP[i][DH : DH + 1, :]
                            )
                        yield
                        # broadcast denominators across partitions via
                        # matmul, then one full-width parallel reciprocal
                        bb = psQ.tile([128, QT], F32, tag="q", name="bb")
                        for i in range(2):
                            nc.tensor.matmul(
                                bb[i * 64 : (i + 1) * 64, :],
                                ones64[:],
                                dn[:, i, :],
                                start=True,
                                stop=True,
                            )
                        rb = wpool.tile(
                            [128, QT], BF16, tag="rb", name="rb", bufs=2
                        )
                        with nc.allow_low_precision(
                            reason="softmax denom ~O(1); bf16 recip ok"
                        ):
                            nc.vector.reciprocal(rb[:], bb[:])
                        yield
                        On = wpool.tile(
                            [128, QT], BF16, tag="On", name="On", bufs=4
                        )
                        nc.vector.tensor_mul(On[:], Or[:], rb[:])
                        On_store[qt][hp] = On
                        yield

                # ---- partial o-proj + x/4 + ReduceScatter (generator) ----
                def oproj_gen(k):
                    for tau in range(CPQ):
                        ti = k * CPQ + tau
                        if tau == 1 and k + 2 < NQ:
                            # prefetch x window for chunk k+2
                            for tj in range((k + 2) * CPQ, (k + 3) * CPQ):
                                load_x(tj)
                        csl = slice(tau * 128, (tau + 1) * 128)
                        ob = wpool.tile([128, D], BF16, tag="ob", name="ob", bufs=2)
                        for nt in range(2):
                            ps = psQ.tile([128, QT], F32, tag="q", name="po")
                            for hp in range(2):
                                nc.tensor.matmul(
                                    ps[:],
                                    On_store[k][hp][:, csl],
                                    ow[hp][:, nt * 512 : (nt + 1) * 512],
                                    start=(hp == 0),
                                    stop=(hp == 1),
                                )
                            nc.vector.scalar_tensor_tensor(
                                ob[:, nt * 512 : (nt + 1) * 512],
                                x_t[ti][:, nt * 512 : (nt + 1) * 512],
                                1.0 / TP,
                                ps[:],
                                ALU.mult,
                                ALU.add,
                            )
                            yield
                        nc.sync.dma_start(
                            rs_in[k][tau * 128 : (tau + 1) * 128, :], ob[:]
                        )
                        yield
                    nc.gpsimd.collective_compute(
                        "ReduceScatter",
                        ALU.add,
                        ins=[rs_in[k][:].opt()],
                        outs=[rs_out[k][:].opt()],
                        replica_groups=groups,
                    )
                    # own-token h for this chunk
                    nc.sync.dma_start(hres[k][:], rs_out[k][:])

                # ---- phase A: chunk 0 stats+norm+qkv, then the other
                # chunks' stats (all Sqrt before the first Exp) ----
                stats_fm(0)
                for _ in chunk_gen(0):
                    pass
                for t4 in range(1, NQ):
                    stats_fm(t4)

                # ---- main loop: attention interleaved with next-chunk
                # norm+qkv and previous-chunk o-proj + ReduceScatter ----
                interleave(attn_gen(0), chunk_gen(1))
                interleave(attn_gen(1), chunk_gen(2), oproj_gen(0))
                interleave(attn_gen(2), chunk_gen(3), oproj_gen(1))
                interleave(attn_gen(3), oproj_gen(2))

                # ---- phase C: norm2 (all Sqrt together) + last o-proj ----
                hnf = fmpool.tile(
                    [128, DC, OWN], BF16, tag="fm", name="hnf", bufs=2
                )
                stats_tile(hres[0], s2_t[0])
                interleave(
                    apply_norm_gen(hres[0], s2_t[0], hnf, 0, on_act=True),
                    oproj_gen(3),
                )
                for k in (1, 2):
                    stats_tile(hres[k], s2_t[k])
                    for _ in apply_norm_gen(
                        hres[k], s2_t[k], hnf, k, on_act=True
                    ):
                        pass
                stats_tile(hres[3], s2_t[3])
                for _ in apply_norm_gen(hres[3], s2_t[3], hnf, 3, on_act=True):
                    pass

            # ---- phase D: MLP (full hidden, own 512 tokens) ----
            # a_fm chunks reuse the SBUF of the (now dead) x / qkw / rope
            # tables
            afm_tags = (
                [f"x{ti}" for ti in range(8)]
                + [f"qkw_t_{i}" for i in range(DC)]
                + [f"cosr_{i}" for i in range(NQ)]
                + [f"sinr_{i}" for i in range(NQ)]
                + [f"vw_m_{i}" for i in range(DC)]
            )
            a_fm = [
                cpool.tile(
                    [128, OWN], BF16, tag=afm_tags[hc], name=f"afm{hc}"
                )
                for hc in range(HCN)
            ]
            with tc.tile_pool(name="psM", bufs=2, space="PSUM") as psM:
                for hc in range(HCN):
                    hsl = slice(hc * 128, (hc + 1) * 128)
                    wg_mc = wpool.tile(
                        [128, DC, 128], BF16, tag="wgs", name="wg_mc", bufs=2
                    )
                    nc.sync.dma_start(wg_mc[:], wgr[:, :, hsl])
                    w1_mc = wpool.tile(
                        [128, DC, 128], BF16, tag="w1s", name="w1_mc", bufs=2
                    )
                    nc.sync.dma_start(w1_mc[:], w1r[:, :, hsl])
                    pg = psM.tile([128, 2, OWN], F32, tag="m", name="pg")
                    for dc in range(DC):
                        nc.tensor.matmul(
                            pg[:, 0, :],
                            wg_mc[:, dc, :],
                            hnf[:, dc, :],
                            start=(dc == 0),
                            stop=(dc == DC - 1),
                        )
                    for dc in range(DC):
                        nc.tensor.matmul(
                            pg[:, 1, :],
                            w1_mc[:, dc, :],
                            hnf[:, dc, :],
                            start=(dc == 0),
                            stop=(dc == DC - 1),
                        )
                    g_sb = wpool.tile(
                        [128, OWN], BF16, tag="g", name="g_sb", bufs=2
                    )
                    nc.scalar.activation(
                        out=g_sb[:], in_=pg[:, 0, :], func=AF.Silu
                    )
                    nc.vector.tensor_mul(a_fm[hc][:], g_sb[:], pg[:, 1, :])

            # ---- phase E: w2 + residual + output ----
            with tc.tile_pool(name="psW", bufs=4, space="PSUM") as psW:
                pws = [
                    psW.tile([128, 2, QT], F32, tag="w", name=f"pw{tt}")
                    for tt in range(NQ)
                ]
                for hc in range(HCN):
                    w2t = wpool.tile(
                        [128, D], BF16, tag="w2t", name="w2t", bufs=2
                    )
                    nc.sync.dma_start(
                        w2t[:], w2w_e[hc * 128 : (hc + 1) * 128, :]
                    )
                    for tt in range(NQ):
                        for ntt in range(2):
                            nc.tensor.matmul(
                                pws[tt][:, ntt, :],
                                a_fm[hc][:, tt * 128 : (tt + 1) * 128],
                                w2t[:, ntt * 512 : (ntt + 1) * 512],
                                start=(hc == 0),
                                stop=(hc == HCN - 1),
                            )
                for tt in range(NQ):
                    outb = wpool.tile(
                        [128, D], F32, tag="outb", name="outb", bufs=1
                    )
                    for ntt in range(2):
                        nc.vector.scalar_tensor_tensor(
                            outb[:, ntt * 512 : (ntt + 1) * 512],
                            hres[tt][:, ntt * 512 : (ntt + 1) * 512],
                            1.0,
                            pws[tt][:, ntt, :],
                            ALU.mult,
                            ALU.add,
                        )
                    nc.gpsimd.dma_start(
                        out_e[tt * 128 : (tt + 1) * 128, :], outb[:]
                    )

    nc.compile()
    return nc


def make_in_maps(x, n1_w, n2_w, qkv_w, o_w, w1_w, wg_w, w2_w, T):
    half = DH // 2
    freqs = np.arange(half, dtype=np.float64) / half
    theta = 1.0 / ROPE_BASE**freqs
    ang = np.arange(T, dtype=np.float64)[:, None] * theta[None, :]  # [T, 32]
    p = np.arange(128) % half
    cosr = np.cos(ang)[:, p].T.astype(BF)  # [128, T]
    sinr = np.sin(ang)[:, p].T.astype(BF)
    tk = np.arange(128)[:, None]
    tq = np.arange(128)[None, :]
    cm = (tq >= tk).astype(BF)  # [128, 128] causal triangle

    ow_t = np.ascontiguousarray(o_w.T)  # [D(f), D(d_out)]
    w1_full = np.ascontiguousarray((w1_w * n2_w[None, :]).T.astype(BF))
    wg_full = np.ascontiguousarray((wg_w * n2_w[None, :]).T.astype(BF))
    w2_full = np.ascontiguousarray(w2_w.T.astype(BF))  # [4096, D]

    in_maps = []
    for c in range(8):
        b, r = c // 4, c % 4
        qs = slice(r * VF, (r + 1) * VF)
        qr = qkv_w[0 * D :][qs] * n1_w[None, :]
        kr = qkv_w[1 * D :][qs] * n1_w[None, :]
        vr = qkv_w[2 * D :][qs] * n1_w[None, :]
        xb = np.asarray(x[b, :T], np.float32)
        in_maps.append(
            {
                "x": np.ascontiguousarray(xb.astype(BF)),
                "x_fm": np.ascontiguousarray(xb.T.astype(BF)),
                "qkw_t": np.ascontiguousarray(
                    np.concatenate([qr, kr], 0).T.astype(BF)
                ),
                "vw_m": np.ascontiguousarray(vr.T.astype(BF)),
                "ow_m": np.ascontiguousarray(ow_t[qs].astype(BF)),
                "w1w_t": w1_full,
                "wgw_t": wg_full,
                "w2w_m": w2_full,
                "cosr": cosr,
                "sinr": sinr,
                "cmask": cm,
                "ident": np.eye(128, dtype=BF),
            }
        )
    return in_maps


_CACHE = {}


def _get_nc(T):
    if T not in _CACHE:
        _CACHE[T] = build_nc(T)
    return _CACHE[T]


def run(inputs, T=2048, trace=False):
    nc = _get_nc(T)
    in_maps = make_in_maps(T=T, **inputs)
    res = run_bass_kernel_spmd(nc, in_maps, core_ids=list(range(8)), trace=trace)
    QT = 512
    NQ = T // QT
    out = np.empty((B, T, D), dtype=np.float32)
    for b in range(B):
        for r in range(TP):
            shard = res.results[b * TP + r]["out"]  # [NQ*128, D]
            for k in range(NQ):
                lo = k * QT + r * 128
                out[b, lo : lo + 128] = shard[k * 128 : (k + 1) * 128]
    return out, res


def kernel(**inputs):
    out, _ = run(inputs, T=2048)
    return out


# revision 32
# speedup vs baseline: 1.0589x; 1.0589x over previous
"""Distributed Trainium2 kernel for a dense transformer block.

Reference computation (per batch):
  x = x + o_proj(attn(rope(qkv(rmsnorm(x))), causal))
  x = x + w2(silu(wg(rmsnorm(x))) * w1(rmsnorm(x)))

Sharding: DP=2 on batch x TP=4 on heads for attention.  After attention each
rank computes the PARTIAL o-proj (its own 256 head-features) for all tokens
of the chunk plus x/4, and a per-chunk ReduceScatter (two 256-token halves)
both completes the sum and hands every rank ownership of 128 tokens per
chunk.  Each rank then runs norm2 + the FULL-hidden MLP on its own 512
tokens and writes the final output shard directly -- only one collective
phase in the whole kernel.

Schedule (v3): fully software-pipelined.
  - rmsnorm statistics for all 16 token tiles are hoisted up front so the ACT
    engine runs only Exp during attention (activation-table switches cost
    1.3us each); the per-chunk normalize+transpose work is interleaved.
  - qkv(qt+1) and o-proj(qt-1) streams are interleaved into the attention(qt)
    score/AV stream so the PE never idles (HAM stays warm).
  - score/exp/AV streams are trimmed to the causal triangle at 128-column
    granularity.
"""

import sys

sys.path.insert(0, "/opt/trn_rl_repo")

import numpy as np
import ml_dtypes

import concourse.bass as bass
import concourse.bacc as bacc
import concourse.mybir as mybir
import concourse.tile as tile
from concourse.bass_utils import run_bass_kernel_spmd

BF = ml_dtypes.bfloat16
F32 = mybir.dt.float32
BF16 = mybir.dt.bfloat16

D = 1024
NH = 16
DH = 64
MULT = 4
EPS = 1e-5
ROPE_BASE = 10000.0
B = 2
TP = 4  # tensor-parallel ranks per group
HPC = NH // TP  # heads per core = 4
QKF = 2 * HPC * DH  # q+k shard features = 512
VF = HPC * DH  # v shard features = 256
MIDF = MULT * D  # full mlp hidden = 4096
HCN = MIDF // 128  # mlp hidden chunks = 32
AF = mybir.ActivationFunctionType
ALU = mybir.AluOpType


def interleave(*gens):
    """Round-robin a set of emission generators until all are exhausted."""
    active = [g for g in gens if g is not None]
    while active:
        nxt = []
        for g in active:
            try:
                next(g)
                nxt.append(g)
            except StopIteration:
                pass
        active = nxt


def delayed(gen, n):
    """Generator wrapper: yield n empty steps before starting gen."""
    for _ in range(n):
        yield
    yield from gen


def paced(gen, p):
    """Spread gen's steps over p rounds each, so a short filler generator
    covers a long main generator instead of exhausting early."""
    while True:
        try:
            next(gen)
        except StopIteration:
            return
        yield
        for _ in range(p - 1):
            yield


def build_nc(T):
    """Build the SPMD graph for one core (token count T per batch)."""
    DC = D // 128  # d chunks = 8
    TT = T // 128  # token tiles = 16
    QT = 512  # q-tile width == collective chunk width
    NQ = T // QT  # 4
    CPQ = QT // 128  # 4
    OWN = NQ * 128  # own tokens per core = 512

    nc = bacc.Bacc("TRN2", target_bir_lowering=False, debug=False, num_devices=8)

    x_e = nc.dram_tensor("x", [T, D], BF16, kind="ExternalInput")
    xf_e = nc.dram_tensor("x_fm", [D, T], BF16, kind="ExternalInput")
    qkw_e = nc.dram_tensor("qkw_t", [D, QKF], BF16, kind="ExternalInput")
    vw_e = nc.dram_tensor("vw_m", [D, VF], BF16, kind="ExternalInput")
    ow_e = nc.dram_tensor("ow_m", [VF, D], BF16, kind="ExternalInput")
    w1w_e = nc.dram_tensor("w1w_t", [D, MIDF], BF16, kind="ExternalInput")
    wgw_e = nc.dram_tensor("wgw_t", [D, MIDF], BF16, kind="ExternalInput")
    w2w_e = nc.dram_tensor("w2w_m", [MIDF, D], BF16, kind="ExternalInput")
    cos_e = nc.dram_tensor("cosr", [128, T], BF16, kind="ExternalInput")
    sin_e = nc.dram_tensor("sinr", [128, T], BF16, kind="ExternalInput")
    cm_e = nc.dram_tensor("cmask", [128, 128], BF16, kind="ExternalInput")
    id_e = nc.dram_tensor("ident", [128, 128], BF16, kind="ExternalInput")
    out_e = nc.dram_tensor("out", [OWN, D], F32, kind="ExternalOutput")

    groups = [[0, 1, 2, 3], [4, 5, 6, 7]]

    with tile.TileContext(nc) as tc:
        with (
            tc.tile_pool(name="const", bufs=1) as cpool,
            tc.tile_pool(name="actfm", bufs=1) as fmpool,
            tc.tile_pool(name="qko", bufs=1) as qkpool,
            tc.tile_pool(name="vaug", bufs=1) as vpool,
            tc.tile_pool(name="xnb", bufs=2) as xnpool,
            tc.tile_pool(name="work", bufs=4) as wpool,
            tc.tile_pool(name="rope", bufs=2) as rpool,
            tc.tile_pool(name="stats", bufs=8) as spool,
            tc.tile_pool(name="hres", bufs=1) as hpool,
            tc.tile_pool(name="dram", bufs=1, space="DRAM") as dpool,
        ):
            # ---- resident weights / tables ----
            def load_tiles(src, width, n, dt=BF16):
                ts = []
                for i in range(n):
                    t = cpool.tile(
                        [128, width], dt, tag=f"{src.name}_{i}", name=f"{src.name}_{i}"
                    )
                    nc.sync.dma_start(t[:], src[i * 128 : (i + 1) * 128, :])
                    ts.append(t)
                return ts

            ident = load_tiles(id_e, 128, 1)[0]
            epsc = cpool.tile([128, 1], F32, tag="epsc", name="epsc")
            nc.vector.memset(epsc[:], EPS)
            onesD = cpool.tile([128, 1], BF16, tag="onesD", name="onesD")
            nc.vector.memset(onesD[:], 1.0)
            ones128r = cpool.tile([1, 128], BF16, tag="ones128r", name="ones128r")
            nc.vector.memset(ones128r[:], 1.0)

            # feature-major x chunks (for norm1); 3 bufs so chunk t4+2's DMA
            # only WAR-waits on chunk t4-1's (already emitted) consumers
            xfr = xf_e.rearrange("(c p) t -> p c t", p=128)
            xf_c = [
                fmpool.tile(
                    [128, DC, QT], BF16, tag="xfm", name=f"xf{t4}", bufs=3
                )
                for t4 in range(NQ)
            ]
            nc.sync.dma_start(xf_c[0][:], xfr[:, :, 0:QT])

            qkw = load_tiles(qkw_e, QKF, DC)
            vw = load_tiles(vw_e, VF, DC)
            ow = load_tiles(ow_e, D, 2)  # local-feature o-proj rows
            w1r = w1w_e.rearrange("(c p) m -> p c m", p=128)
            wgr = wgw_e.rearrange("(c p) m -> p c m", p=128)
            cos_t, sin_t = [], []
            for t4 in range(NQ):
                for src, dst in ((cos_e, cos_t), (sin_e, sin_t)):
                    t = cpool.tile(
                        [128, QT], BF16, tag=f"{src.name}_{t4}",
                        name=f"{src.name}c{t4}",
                    )
                    nc.sync.dma_start(t[:], src[:, t4 * QT : (t4 + 1) * QT])
                    dst.append(t)
            tri = load_tiles(cm_e, 128, 1)[0]
            ones64 = cpool.tile([1, 64], BF16, tag="ones64", name="ones64")
            nc.vector.memset(ones64[:], 1.0)

            # remaining feature-major x chunks
            for t4 in range(1, NQ):
                nc.sync.dma_start(
                    xf_c[t4][:], xfr[:, :, t4 * QT : (t4 + 1) * QT]
                )
            # token-major x (for the o-proj residual): 8-tile sliding
            # window; chunks 2-3 are prefetched from inside oproj_gen
            x_t = [None] * TT

            def load_x(ti):
                t = cpool.tile(
                    [128, D], BF16, tag=f"x{ti % 8}", name=f"x{ti}"
                )
                nc.sync.dma_start(t[:], x_e[ti * 128 : (ti + 1) * 128, :])
                x_t[ti] = t

            for ti in range(8):
                load_x(ti)

            rs_in = [
                dpool.tile([QT, D], BF16, name=f"rs_in{k}") for k in range(NQ)
            ]
            rs_out = [
                dpool.tile([QT // TP, D], BF16, name=f"rs_out{k}")
                for k in range(NQ)
            ]

            # ---- persistent activation tiles ----
            # normalized x, feature-major: xnf[t4][:, dc, tau*128:...] holds
            # (x-hat chunk)^T for d-block dc
            xnf_c = [
                fmpool.tile(
                    [128, DC, QT], BF16, tag="fm", name=f"xnf{t4}", bufs=2
                )
                for t4 in range(NQ)
            ]
            q_sb = [
                qkpool.tile([128, T], BF16, tag=f"qk{i}", name=f"q{i}")
                for i in range(2)
            ]
            k_sb = [
                qkpool.tile([128, T], BF16, tag=f"qk{i + 2}", name=f"k{i}")
                for i in range(2)
            ]
            v_aug = [
                vpool.tile([128, HPC, DH + 1], BF16, tag=f"va{ti}", name=f"va{ti}")
                for ti in range(TT)
            ]
            hres = [
                hpool.tile([128, D], BF16, tag=f"hr{k}", name=f"hr{k}")
                for k in range(NQ)
            ]
            # normalized attention outputs per chunk+headpair (o-proj lhsT)
            On_store = [[None, None] for _ in range(NQ)]
            # norm1 1/rms rows (per chunk) + norm2 scalars (per chunk)
            s1r_t = [
                cpool.tile([1, QT], BF16, tag=f"s1r{t4}", name=f"s1r{t4}")
                for t4 in range(NQ)
            ]
            s2_t = [
                cpool.tile([128, 1], F32, tag=f"s2_{k}", name=f"s2_{k}")
                for k in range(NQ)
            ]

            def stats_tile(xt, s1):
                """rms stats for one token tile -> s1 = 1/sqrt(mean sq+eps)."""
                ss = spool.tile([128, 1], F32, tag="ss", name="ss")
                sq = xnpool.tile([128, D], BF16, tag="sq", name="sq", bufs=1)
                nc.vector.scalar_tensor_tensor(
                    sq[:], xt[:], 1.0, xt[:], ALU.mult, ALU.mult, accum_out=ss[:]
                )
                sr = spool.tile([128, 1], F32, tag="sr", name="sr")
                nc.scalar.activation(
                    out=sr[:], in_=ss[:], func=AF.Sqrt, bias=epsc[:], scale=1.0 / D
                )
                nc.vector.reciprocal(s1[:], sr[:])

            eps1 = spool.tile([1, 1], F32, tag="eps1", name="eps1")
            nc.vector.memset(eps1[:], EPS)

            with (
                tc.tile_pool(name="psSC", bufs=2, space="PSUM") as psSC,
                tc.tile_pool(name="psAV", bufs=2, space="PSUM") as psAV,
                tc.tile_pool(name="psQ", bufs=2, space="PSUM") as psQ,
            ):

                def apply_norm_gen(xt, s1, fm_c, tau, on_act=False, pool=None):
                    """normalize token tile and write feature-major block."""
                    pool_, tag_ = (psQ, "q") if pool is None else pool
                    xn = xnpool.tile([128, D], BF16, tag="xn", name="xn", bufs=1)
                    nc.vector.tensor_scalar_mul(xn[:], xt[:], s1[:])
                    for di in range(DC):
                        tp = pool_.tile([128, 128], BF16, tag=tag_, name="tp")
                        nc.tensor.transpose(
                            tp[:], xn[:, di * 128 : (di + 1) * 128], ident[:]
                        )
                        dst = fm_c[:, di, tau * 128 : (tau + 1) * 128]
                        if on_act:
                            nc.scalar.copy(dst, tp[:])
                        else:
                            nc.vector.tensor_copy(dst, tp[:])
                        if di % 4 == 3:
                            yield

                # ---- qkv + rope + v for one chunk (generator) ----
                def qkv_gen(t4):
                    tsl = slice(t4 * QT, (t4 + 1) * QT)
                    for m in range(4):  # q01 q23 k01 k23
                        dst = q_sb[m] if m < 2 else k_sb[m - 2]
                        ps = psQ.tile([128, QT], F32, tag="q", name="ps")
                        for dc in range(DC):
                            nc.tensor.matmul(
                                ps[:],
                                qkw[dc][:, m * 128 : (m + 1) * 128],
                                xnf_c[t4][:, dc, :],
                                start=(dc == 0),
                                stop=(dc == DC - 1),
                            )
                            if dc == 3:
                                yield
                        qb = rpool.tile([128, QT], BF16, tag="qb", name="qb")
                        nc.vector.tensor_copy(qb[:], ps[:])
                        rot = rpool.tile([128, QT], BF16, tag="rot", name="rot")
                        for hb in (0, 64):
                            nc.vector.tensor_scalar_mul(
                                rot[hb : hb + 32, :], qb[hb + 32 : hb + 64, :], -1.0
                            )
                            nc.vector.tensor_copy(
                                rot[hb + 32 : hb + 64, :], qb[hb : hb + 32, :]
                            )
                        t1 = rpool.tile([128, QT], BF16, tag="t1", name="t1")
                        nc.vector.tensor_mul(t1[:], qb[:], cos_t[t4][:])
                        t2 = rpool.tile([128, QT], BF16, tag="t2", name="t2")
                        nc.vector.tensor_mul(t2[:], rot[:], sin_t[t4][:])
                        nc.vector.tensor_add(dst[:, tsl], t1[:], t2[:])
                        yield
                    for tau in range(CPQ):
                        ti = t4 * CPQ + tau
                        ps = psQ.tile([128, VF], F32, tag="q", name="psv")
                        for dc in range(DC):
                            nc.tensor.matmul(
                                ps[:],
                                xnf_c[t4][:, dc, tau * 128 : (tau + 1) * 128],
                                vw[dc][:],
                                start=(dc == 0),
                                stop=(dc == DC - 1),
                            )
                        va = v_aug[ti]
                        nc.vector.tensor_copy(
                            va[:, :, 0:DH], ps.rearrange("p (h d) -> p h d", h=HPC)
                        )
                        nc.vector.memset(va[:, :, DH : DH + 1], 1.0)
                        yield

                def stats_fm(t4):
                    """norm1 stats for one chunk from feature-major x:
                    s1r[t4][0, t] = 1/sqrt(mean_d x[t,d]^2 + eps)."""
                    xsq = xnpool.tile(
                        [128, DC, QT], BF16, tag="xsq", name="xsq", bufs=1
                    )
                    nc.vector.tensor_mul(xsq[:], xf_c[t4][:], xf_c[t4][:])
                    ssr = psQ.tile([1, QT], F32, tag="q", name="ssr")
                    for dc in range(DC):
                        nc.tensor.matmul(
                            ssr[:],
                            onesD[:],
                            xsq[:, dc, :],
                            start=(dc == 0),
                            stop=(dc == DC - 1),
                        )
                    srr = spool.tile([1, QT], F32, tag="srr", name="srr", bufs=2)
                    nc.scalar.activation(
                        out=srr[:], in_=ssr[:], func=AF.Sqrt, bias=eps1[:],
                        scale=1.0 / D,
                    )
                    with nc.allow_low_precision(reason="1/rms in bf16"):
                        nc.vector.reciprocal(s1r_t[t4][:], srr[:])

                def chunk_gen(t4):
                    """norm1-apply (via row broadcast) then qkv for chunk
                    t4 (chained)."""
                    bc = psQ.tile([128, QT], F32, tag="q", name="bc")
                    nc.tensor.matmul(
                        bc[:], ones128r[:], s1r_t[t4][:], start=True, stop=True
                    )
                    bcs = xnpool.tile([128, QT], BF16, tag="bcs", name="bcs", bufs=2)
                    nc.vector.tensor_copy(bcs[:], bc[:])
                    yield
                    for dc in range(DC):
                        nc.vector.tensor_mul(
                            xnf_c[t4][:, dc, :], xf_c[t4][:, dc, :], bcs[:]
                        )
                        if dc % 4 == 3:
                            yield
                    yield from qkv_gen(t4)

                # ---- attention for one q-chunk (generator) ----
                def attn_gen(qt):
                    ncks = CPQ * (qt + 1)
                    for hp in range(2):
                        opsP = [
                            psAV.tile([DH + 1, QT], F32, tag="av", name=f"ops{i}")
                            for i in range(2)
                        ]

                        def emit_scores(ck):
                            j = ck - CPQ * qt  # >=0 inside the diagonal block
                            lo = max(j, 0) * 128
                            sp = psSC.tile([128, 2, QT], F32, tag="sc", name="sp")
                            for i in range(2):
                                hb = i * 64
                                nc.tensor.matmul(
                                    sp[:, i, lo:QT],
                                    k_sb[hp][
                                        hb : hb + DH, ck * 128 : (ck + 1) * 128
                                    ],
                                    q_sb[hp][
                                        hb : hb + DH,
                                        qt * QT + lo : (qt + 1) * QT,
                                    ],
                                    start=True,
                                    stop=True,
                                )
                            pt = wpool.tile(
                                [128, 2, QT], BF16, tag="pt", name="pt", bufs=5
                            )
                            nc.scalar.activation(
                                out=pt[:, :, lo:],
                                in_=sp[:, :, lo:],
                                func=AF.Exp,
                                scale=0.125,
                            )
                            if j >= 0:
                                for i in range(2):
                                    nc.vector.tensor_mul(
                                        pt[:, i, lo : lo + 128],
                                        pt[:, i, lo : lo + 128],
                                        tri[:],
                                    )
                            return pt, lo

                        def emit_av(ck, pt_lo):
                            pt, lo = pt_lo
                            for i in range(2):
                                nc.tensor.matmul(
                                    opsP[i][:, lo:QT],
                                    v_aug[ck][:, 2 * hp + i, :],
                                    pt[:, i, lo:],
                                    start=(ck == 0),
                                    stop=(ck == ncks - 1),
                                )

                        # scores emitted one chunk ahead of AV so the PE
                        # never waits on the Exp chain
                        prev = emit_scores(0)
                        yield
                        for ck in range(1, ncks):
                            cur = emit_scores(ck)
                            emit_av(ck - 1, prev)
                            prev = cur
                            yield
                        emit_av(ncks - 1, prev)
                        yield
                        # copy raw head outputs + denominator rows out of
                        # PSUM fast, freeing the AV accumulators; both heads
                        # stack into one 128-partition tile so the later
                        # SBUF-only ops have uniform start partitions
                        Or = wpool.tile(
                            [128, QT], BF16, tag="Or", name="Or", bufs=4
                        )
                        dn = spool.tile(
                            [1, 2, QT], BF16, tag="dn", name="dn", bufs=2
                        )
                        for i in range(2):
                            nc.vector.tensor_copy(
                                Or[i * 64 : (i + 1) * 64, :], opsP[i][0:DH, :]
                            )
                            nc.vector.tensor_copy(
                                dn[:, i, :], opsP[i][DH : DH + 1, :]
                            )
                        yield
                        # broadcast denominators across partitions via
                        # matmul, then one full-width parallel reciprocal
                        bb = psQ.tile([128, QT], F32, tag="q", name="bb")
                        for i in range(2):
                            nc.tensor.matmul(
                                bb[i * 64 : (i + 1) * 64, :],
                                ones64[:],
                                dn[:, i, :],
                                start=True,
                                stop=True,
                            )
                        rb = wpool.tile(
                            [128, QT], BF16, tag="rb", name="rb", bufs=2
                        )
                        with nc.allow_low_precision(
                            reason="softmax denom ~O(1); bf16 recip ok"
                        ):
                            nc.vector.reciprocal(rb[:], bb[:])
                        yield
                        On = wpool.tile(
                            [128, QT], BF16, tag="On", name="On", bufs=4
                        )
                        nc.vector.tensor_mul(On[:], Or[:], rb[:])
                        On_store[qt][hp] = On
                        yield

                # ---- partial o-proj + x/4 + ReduceScatter (generator) ----
                def oproj_gen(k):
                    for tau in range(CPQ):
                        ti = k * CPQ + tau
                        if tau == 1 and k + 2 < NQ:
                            # prefetch x window for chunk k+2
                            for tj in range((k + 2) * CPQ, (k + 3) * CPQ):
                                load_x(tj)
                        csl = slice(tau * 128, (tau + 1) * 128)
                        ob = wpool.tile([128, D], BF16, tag="ob", name="ob", bufs=2)
                        for nt in range(2):
                            ps = psQ.tile([128, QT], F32, tag="q", name="po")
                            for hp in range(2):
                                nc.tensor.matmul(
                                    ps[:],
                                    On_store[k][hp][:, csl],
                                    ow[hp][:, nt * 512 : (nt + 1) * 512],
                                    start=(hp == 0),
                                    stop=(hp == 1),
                                )
                            nc.vector.scalar_tensor_tensor(
                                ob[:, nt * 512 : (nt + 1) * 512],
                                x_t[ti][:, nt * 512 : (nt + 1) * 512],
                                1.0 / TP,
                                ps[:],
                                ALU.mult,
                                ALU.add,
                            )
                            yield
                        nc.sync.dma_start(
                            rs_in[k][tau * 128 : (tau + 1) * 128, :], ob[:]
                        )
                        yield
                    nc.gpsimd.collective_compute(
                        "ReduceScatter",
                        ALU.add,
                        ins=[rs_in[k][:].opt()],
                        outs=[rs_out[k][:].opt()],
                        replica_groups=groups,
                    )
                    # own-token h for this chunk -- on the gpsimd queue:
                    # this DMA waits on the ReduceScatter, and on the shared
                    # sync queue that wait head-of-line-blocks every later
                    # DMA (x prefetches, rs staging), stalling the whole
                    # attention pipeline
                    nc.gpsimd.dma_start(hres[k][:], rs_out[k][:])

                # ---- phase A: chunk 0 stats+norm+qkv, then the other
                # chunks' stats (all Sqrt before the first Exp) ----
                stats_fm(0)
                for _ in chunk_gen(0):
                    pass
                for t4 in range(1, NQ):
                    stats_fm(t4)

                # ---- main loop: attention interleaved with next-chunk
                # norm+qkv and previous-chunk o-proj + ReduceScatter ----
                interleave(attn_gen(0), chunk_gen(1))
                interleave(attn_gen(1), chunk_gen(2), paced(oproj_gen(0), 2))
                interleave(attn_gen(2), chunk_gen(3), paced(oproj_gen(1), 2))
                interleave(attn_gen(3), paced(oproj_gen(2), 3))

                # ---- phase C: norm2 (all Sqrt together) + last o-proj ----
                hnf = fmpool.tile(
                    [128, DC, OWN], BF16, tag="fm", name="hnf", bufs=2
                )
                stats_tile(hres[0], s2_t[0])
                interleave(
                    apply_norm_gen(hres[0], s2_t[0], hnf, 0, on_act=True),
                    oproj_gen(3),
                )
                for k in (1, 2):
                    stats_tile(hres[k], s2_t[k])
                    for _ in apply_norm_gen(
                        hres[k], s2_t[k], hnf, k, on_act=True
                    ):
                        pass

            # ---- phase D: MLP (full hidden, own 512 tokens) ----
            # a_fm chunks reuse the SBUF of the (now dead) x / qkw / rope
            # tables
            afm_tags = (
                [f"x{ti}" for ti in range(8)]
                + [f"qkw_t_{i}" for i in range(DC)]
                + [f"cosr_{i}" for i in range(NQ)]
                + [f"sinr_{i}" for i in range(NQ)]
                + [f"vw_m_{i}" for i in range(DC)]
            )
            a_fm = [
                cpool.tile(
                    [128, OWN], BF16, tag=afm_tags[hc], name=f"afm{hc}"
                )
                for hc in range(HCN)
            ]
            KS = 6  # hidden chunks run early on chunks 0-2's tokens
            NT3 = 3 * 128  # columns owned by chunks 0-2

            with tc.tile_pool(name="psM", bufs=2, space="PSUM") as psM:

                def mlp_hc(hc, lo, hi, tag_sfx=""):
                    hsl = slice(hc * 128, (hc + 1) * 128)
                    wg_mc = wpool.tile(
                        [128, DC, 128], BF16, tag="wgs", name="wg_mc", bufs=2
                    )
                    nc.sync.dma_start(wg_mc[:], wgr[:, :, hsl])
                    w1_mc = wpool.tile(
                        [128, DC, 128], BF16, tag="w1s", name="w1_mc", bufs=2
                    )
                    nc.sync.dma_start(w1_mc[:], w1r[:, :, hsl])
                    pg = psM.tile([128, 2, OWN], F32, tag="m", name="pg")
                    for half in range(2):
                        for dc in range(DC):
                            nc.tensor.matmul(
                                pg[:, half, lo:hi],
                                (wg_mc if half == 0 else w1_mc)[:, dc, :],
                                hnf[:, dc, lo:hi],
                                start=(dc == 0),
                                stop=(dc == DC - 1),
                            )
                    g_sb = wpool.tile(
                        [128, OWN], BF16, tag="g", name="g_sb", bufs=2
                    )
                    nc.scalar.activation(
                        out=g_sb[:, lo:hi], in_=pg[:, 0, lo:hi], func=AF.Silu
                    )
                    nc.vector.tensor_mul(
                        a_fm[hc][:, lo:hi], g_sb[:, lo:hi], pg[:, 1, lo:hi]
                    )

                # early pass: chunks 0-2's token columns only -- this PE
                # work runs while chunk 3's ReduceScatter is in flight
                for hc in range(KS):
                    mlp_hc(hc, 0, NT3)
                # chunk 3's h lands -> norm2(3) (transposes via psM slots)
                stats_tile(hres[3], s2_t[3])
                for _ in apply_norm_gen(
                    hres[3], s2_t[3], hnf, 3, on_act=True, pool=(psM, "m")
                ):
                    pass
                for hc in range(KS, HCN):
                    mlp_hc(hc, 0, OWN)
                # deferred: chunk-3 columns of the early hidden chunks
                for hc in range(KS):
                    mlp_hc(hc, NT3, OWN)

            # ---- phase E: w2 + residual + output ----
            with tc.tile_pool(name="psW", bufs=4, space="PSUM") as psW:
                pws = [
                    psW.tile([128, 2, QT], F32, tag="w", name=f"pw{tt}")
                    for tt in range(NQ)
                ]
                for hc in range(HCN):
                    w2t = wpool.tile(
                        [128, D], BF16, tag="w2t", name="w2t", bufs=2
                    )
                    nc.sync.dma_start(
                        w2t[:], w2w_e[hc * 128 : (hc + 1) * 128, :]
                    )
                    for tt in range(NQ):
                        for ntt in range(2):
                            nc.tensor.matmul(
                                pws[tt][:, ntt, :],
                                a_fm[hc][:, tt * 128 : (tt + 1) * 128],
                                w2t[:, ntt * 512 : (ntt + 1) * 512],
                                start=(hc == 0),
                                stop=(hc == HCN - 1),
                            )
                for tt in range(NQ):
                    outb = wpool.tile(
                        [128, D], F32, tag="outb", name="outb", bufs=1
                    )
                    for ntt in range(2):
                        nc.vector.scalar_tensor_tensor(
                            outb[:, ntt * 512 : (ntt + 1) * 512],
                            hres[tt][:, ntt * 512 : (ntt + 1) * 512],
                            1.0,
                            pws[tt][:, ntt, :],
                            ALU.mult,
                            ALU.add,
                        )
                    nc.gpsimd.dma_start(
                        out_e[tt * 128 : (tt + 1) * 128, :], outb[:]
                    )

    nc.compile()
    return nc


def make_in_maps(x, n1_w, n2_w, qkv_w, o_w, w1_w, wg_w, w2_w, T):
    half = DH // 2
    freqs = np.arange(half, dtype=np.float64) / half
    theta = 1.0 / ROPE_BASE**freqs
    ang = np.arange(T, dtype=np.float64)[:, None] * theta[None, :]  # [T, 32]
    p = np.arange(128) % half
    cosr = np.cos(ang)[:, p].T.astype(BF)  # [128, T]
    sinr = np.sin(ang)[:, p].T.astype(BF)
    tk = np.arange(128)[:, None]
    tq = np.arange(128)[None, :]
    cm = (tq >= tk).astype(BF)  # [128, 128] causal triangle

    ow_t = np.ascontiguousarray(o_w.T)  # [D(f), D(d_out)]
    w1_full = np.ascontiguousarray((w1_w * n2_w[None, :]).T.astype(BF))
    wg_full = np.ascontiguousarray((wg_w * n2_w[None, :]).T.astype(BF))
    w2_full = np.ascontiguousarray(w2_w.T.astype(BF))  # [4096, D]

    in_maps = []
    for c in range(8):
        b, r = c // 4, c % 4
        qs = slice(r * VF, (r + 1) * VF)
        qr = qkv_w[0 * D :][qs] * n1_w[None, :]
        kr = qkv_w[1 * D :][qs] * n1_w[None, :]
        vr = qkv_w[2 * D :][qs] * n1_w[None, :]
        xb = np.asarray(x[b, :T], np.float32)
        in_maps.append(
            {
                "x": np.ascontiguousarray(xb.astype(BF)),
                "x_fm": np.ascontiguousarray(xb.T.astype(BF)),
                "qkw_t": np.ascontiguousarray(
                    np.concatenate([qr, kr], 0).T.astype(BF)
                ),
                "vw_m": np.ascontiguousarray(vr.T.astype(BF)),
                "ow_m": np.ascontiguousarray(ow_t[qs].astype(BF)),
                "w1w_t": w1_full,
                "wgw_t": wg_full,
                "w2w_m": w2_full,
                "cosr": cosr,
                "sinr": sinr,
                "cmask": cm,
                "ident": np.eye(128, dtype=BF),
            }
        )
    return in_maps


_CACHE = {}


def _get_nc(T):
    if T not in _CACHE:
        _CACHE[T] = build_nc(T)
    return _CACHE[T]


def run(inputs, T=2048, trace=False):
    nc = _get_nc(T)
    in_maps = make_in_maps(T=T, **inputs)
    res = run_bass_kernel_spmd(nc, in_maps, core_ids=list(range(8)), trace=trace)
    QT = 512
    NQ = T // QT
    out = np.empty((B, T, D), dtype=np.float32)
    for b in range(B):
        for r in range(TP):
            shard = res.results[b * TP + r]["out"]  # [NQ*128, D]
            for k in range(NQ):
                lo = k * QT + r * 128
                out[b, lo : lo + 128] = shard[k * 128 : (k + 1) * 128]
    return out, res


def kernel(**inputs):
    out, _ = run(inputs, T=2048)
    return out
